# revision 1
# baseline (speedup 1.0000x reference)
"""GQA attention kernel for Trainium2, tensor-parallel over heads across 8 cores.

Problem: T=2048, D=4096, H=32 q-heads, G=8 kv-heads, HD=128.
Per core: 4 q heads + 1 kv head (group), full T.
  q = rmsnorm(x @ Wq_c) -> rope -> qT [HD, T]
  k = rmsnorm(x @ Wk_c) -> rope -> kT [HD, T]; v natural [T, HD]
  scoresT[j,i] blocks = kT_j.T @ qT (f32r matmuls), causal-blocked
  expT = exp(scoresT/sqrt(HD) + diag mask); denom via ones-matmul
  attnT[HD, T] += v_j.T @ expT_j; normalized by 1/denom
  out_partial = attnT.T @ Wo_c (bf16, row-sharded) -> host sums the 8 partials.
"""
import sys

sys.path.insert(0, '/opt/trn_rl_repo')

import numpy as np
import ml_dtypes

import concourse.bass as bass
import concourse.bacc as bacc
import concourse.mybir as mybir
import concourse.tile as tile
from concourse.bass_utils import run_bass_kernel_spmd

F32 = mybir.dt.float32
F32R = mybir.dt.float32r
BF16 = mybir.dt.bfloat16
AF = mybir.ActivationFunctionType
OP = mybir.AluOpType

T = 2048
D = 4096
H = 32
G = 8
HD = 128
NCORES = 8
HPC = H // NCORES          # 4 q heads per core
NB = T // 128              # 16 row/col blocks
NSUP = NB // 4             # 4 supertiles of 512 queries
DKT = D // 128             # 32 contraction tiles for projections
NOC = D // 512             # 8 out-proj column blocks
EPS = 1e-6
ISQ = 1.0 / float(np.sqrt(HD))
NEG = -1.0e30


def _rotview(ap):
    """[128, 128] AP -> [128, 2, 64] view reading cols 64:128 then 0:64."""
    return bass.AP(ap.tensor, ap.offset + 64, [list(ap.ap[0]), [-64, 2], [1, 64]])


def _emit(nc, tc):
    xt = nc.dram_tensor("xt", [NB, 128, DKT * 128], F32R, kind="ExternalInput")
    wq = nc.dram_tensor("wq", [128, DKT * 512], F32R, kind="ExternalInput")
    wkv = nc.dram_tensor("wkv", [128, DKT * 256], F32R, kind="ExternalInput")
    wo = nc.dram_tensor("wo", [128, HPC * NOC * 512], F32R, kind="ExternalInput")
    tbl = nc.dram_tensor("tbl", [4, T, HD], F32, kind="ExternalInput")
    tri01 = nc.dram_tensor("tri01", [128, 128], F32, kind="ExternalInput")
    ztri01 = nc.dram_tensor("ztri01", [128, 256], F32, kind="ExternalInput")
    onescol = nc.dram_tensor("onescol", [128, 1], F32R, kind="ExternalInput")
    ident = nc.dram_tensor("ident", [128, 128], F32R, kind="ExternalInput")
    out = nc.dram_tensor("out", [T, D], F32, kind="ExternalOutput")

    import contextlib
    ctx = contextlib.ExitStack()
    with ctx:
        const_p = ctx.enter_context(tc.tile_pool(name="const", bufs=1))
        tri_sb = const_p.tile([128, 128], F32)
        ztri_sb = const_p.tile([128, 256], F32)
        oc_sb = const_p.tile([128, 1], F32R)
        id_sb = const_p.tile([128, 128], F32R)
        epsb = const_p.tile([128, 1], F32)

        def load_consts():
            nc.sync.dma_start(id_sb[:], ident.ap())
            nc.sync.dma_start(tri_sb[:], tri01.ap())
            nc.sync.dma_start(ztri_sb[:], ztri01.ap())
            nc.sync.dma_start(oc_sb[:], onescol.ap())
            nc.vector.memset(epsb[:], float(HD) * EPS)

        # persistent SBUF: kT, v, attnT, Wo (bf16, preloaded during phase 1)
        pers = ctx.enter_context(tc.tile_pool(name="pers", bufs=1))
        kT = pers.tile([128, T], F32R)
        v_sb = pers.tile([128, NB, 128], F32R)
        attnT = pers.tile([128, HPC, T], F32R)

        # DRAM scratch for qT spill
        dram_p = ctx.enter_context(tc.tile_pool(name="dram", bufs=1, space="DRAM"))
        qt_d = dram_p.tile([HPC, 128, T], F32R)

        # right-side psum shared by phase-1 transposes and phase-2 scores
        big_p = ctx.enter_context(
            tc.tile_pool(name="big_p", bufs=3, space="PSUM", side="right"))
        pat_p = ctx.enter_context(
            tc.tile_pool(name="pat_p", bufs=2, space="PSUM", side="right"))

        # ---------------- Phase 1: projections + rmsnorm + rope + transpose ----
        with tc.tile_pool(name="wq_p", bufs=1) as wq_p, \
             tc.tile_pool(name="wkv_p", bufs=1) as wkv_p, \
             tc.tile_pool(name="xb_p", bufs=2) as xb_p, \
             tc.tile_pool(name="tbl_p", bufs=2) as tbl_p, \
             tc.tile_pool(name="scr_p", bufs=2) as scr_p, \
             tc.tile_pool(name="stg_p", bufs=2) as stg_p, \
             tc.tile_pool(name="psq_p", bufs=2, space="PSUM") as psq_p, \
             tc.tile_pool(name="pskv_p", bufs=1, space="PSUM") as pskv_p:
            wq_sb = wq_p.tile([128, DKT * 512], F32R)
            wkv_sb = wkv_p.tile([128, DKT * 256], F32R)
            def load_wq(part):
                if part == 0:
                    nc.sync.dma_start(wq_sb[:, 0:1024], wq.ap()[:, 0:1024])
                    return
                for chh in range(1, 16):
                    w = DKT * 512 // 16
                    nc.sync.dma_start(wq_sb[:, chh * w:(chh + 1) * w],
                                      wq.ap()[:, chh * w:(chh + 1) * w])
                for chh in range(4):
                    w = DKT * 256 // 4
                    nc.sync.dma_start(wkv_sb[:, chh * w:(chh + 1) * w],
                                      wkv.ap()[:, chh * w:(chh + 1) * w])
            wq3 = wq_sb[:].rearrange("p (k n) -> p k n", k=DKT)
            wkv3 = wkv_sb[:].rearrange("p (k n) -> p k n", k=DKT)

            for i in range(NB):
                xb = xb_p.tile([128, DKT * 128], F32R)
                if i == 0:
                    load_wq(0)
                nch = 8 if i == 0 else 2
                for chh in range(nch):
                    w = DKT * 128 // nch
                    nc.sync.dma_start(xb[:, chh * w:(chh + 1) * w],
                                      xt.ap()[i][:, chh * w:(chh + 1) * w])
                xb3 = xb[:].rearrange("p (k n) -> p k n", k=DKT)

                tb = tbl_p.tile([128, 4, HD], F32, tag="tb")
                nc.sync.dma_start(
                    tb[:],
                    tbl.ap()[:, i * 128:(i + 1) * 128, :].transpose([1, 0, 2]))
                cq, sq, ck, sk = (tb[:, 0, :], tb[:, 1, :], tb[:, 2, :],
                                  tb[:, 3, :])
                if i == 0:
                    load_wq(1)
                    load_consts()

                psq = psq_p.tile([128, 512], F32)
                pskv = pskv_p.tile([128, 256], F32)
                for kk in range(DKT):
                    nc.tensor.matmul(psq[:], xb3[:, kk, :], wq3[:, kk, :],
                                     start=(kk == 0), stop=(kk == DKT - 1))
                for kk in range(DKT):
                    nc.tensor.matmul(pskv[:], xb3[:, kk, :], wkv3[:, kk, :],
                                     start=(kk == 0), stop=(kk == DKT - 1))

                # rms stats for 4 q chunks + 1 k chunk
                ssq = scr_p.tile([128, 8], F32, tag="ssq")
                sqscr = scr_p.tile([128, 128], F32, tag="sqscr")
                for c in range(HPC):
                    nc.scalar.activation(sqscr[:], psq[:, c * 128:(c + 1) * 128],
                                         AF.Square, accum_out=ssq[:, c:c + 1])
                nc.scalar.activation(sqscr[:], pskv[:, 0:128],
                                     AF.Square, accum_out=ssq[:, 4:5])
                rstd = scr_p.tile([128, 8], F32, tag="rstd")
                nc.scalar.activation(rstd[:, 0:5], ssq[:, 0:5], AF.Sqrt,
                                     bias=epsb[:])
                nc.vector.reciprocal_approx_fast(rstd[:, 0:5], rstd[:, 0:5])

                # rope (rmsnorm scale fused; sqrt(HD) and qn/kn weights folded
                # into the host-side cos/sin tables)
                roq = scr_p.tile([128, 512], F32R, tag="roq")
                rok = scr_p.tile([128, 128], F32R, tag="rok")
                t1 = scr_p.tile([128, 128], F32, tag="t1")
                t2 = scr_p.tile([128, 128], F32, tag="t2")
                for c in range(HPC):
                    ch = psq[:, c * 128:(c + 1) * 128]
                    nc.vector.scalar_tensor_tensor(
                        t1[:], ch, rstd[:, c:c + 1], cq,
                        op0=OP.mult, op1=OP.mult)
                    nc.vector.scalar_tensor_tensor(
                        t2[:].rearrange("p (a b) -> p a b", a=2), _rotview(ch),
                        rstd[:, c:c + 1], sq.rearrange("p (a b) -> p a b", a=2),
                        op0=OP.mult, op1=OP.mult)
                    nc.vector.tensor_add(roq[:, c * 128:(c + 1) * 128], t1[:], t2[:])
                chk = pskv[:, 0:128]
                nc.vector.scalar_tensor_tensor(
                    t1[:], chk, rstd[:, 4:5], ck, op0=OP.mult, op1=OP.mult)
                nc.vector.scalar_tensor_tensor(
                    t2[:].rearrange("p (a b) -> p a b", a=2), _rotview(chk),
                    rstd[:, 4:5], sk.rearrange("p (a b) -> p a b", a=2),
                    op0=OP.mult, op1=OP.mult)
                nc.vector.tensor_add(rok[:], t1[:], t2[:])

                # v: psum -> sbuf
                nc.scalar.copy(v_sb[:, i, :], pskv[:, 128:256])

                # transposes: 4 q chunks + 1 k via shared right-side psum pool
                trq = big_p.tile([128, 512], F32, tag="big", name="trq")
                for c in range(HPC):
                    nc.tensor.transpose(
                        trq[:, c * 128:(c + 1) * 128].bitcast(F32R),
                        roq[:, c * 128:(c + 1) * 128], id_sb[:])
                trk = big_p.tile([128, 512], F32, tag="big", name="trk")
                nc.tensor.transpose(trk[:, 0:128].bitcast(F32R), rok[:], id_sb[:])

                qstg = stg_p.tile([128, 512], F32R)
                nc.scalar.copy(qstg[:], trq[:])
                nc.gpsimd.dma_start(
                    qt_d[:, :, i * 128:(i + 1) * 128].transpose([1, 0, 2]),
                    qstg[:].rearrange("p (h n) -> p h n", h=HPC))
                nc.scalar.copy(kT[:, i * 128:(i + 1) * 128], trk[:, 0:128])

        # ---------------- Phase 2+3: attention + out-proj, interleaved --------
        with tc.tile_pool(name="wo_p", bufs=1) as wo_p, \
             tc.tile_pool(name="qt_p", bufs=2, side="right") as qt_p, \
             tc.tile_pool(name="exp_p", bufs=3, side="right") as exp_p, \
             tc.tile_pool(name="rc_p", bufs=2) as rc_p, \
             tc.tile_pool(name="pden_p", bufs=1, space="PSUM") as pden_p, \
             tc.tile_pool(name="po_p", bufs=2, space="PSUM") as po_p, \
             tc.tile_pool(name="ost_p", bufs=2) as ost_p:
            wo_sb = wo_p.tile([128, HPC * NOC * 512], F32R)
            for chh in range(8):
                w = HPC * NOC * 512 // 8
                nc.sync.dma_start(wo_sb[:, chh * w:(chh + 1) * w],
                                  wo.ap()[:, chh * w:(chh + 1) * w])
            wo4 = wo_sb[:].rearrange("p (h n c) -> p h n c", h=HPC, n=NOC)
            for g in range(NSUP):
                for h in range(HPC):
                    qtile = qt_p.tile([128, 512], F32R)
                    nc.gpsimd.dma_start(qtile[:], qt_d[h, :, g * 512:(g + 1) * 512])
                    pat = pat_p.tile([128, 512], F32, tag="pat", name="pat")
                    pden = pden_p.tile([1, 512], F32)
                    nj = 4 * g + 4
                    for j in range(nj):
                        cd = max(0, j - 4 * g)
                        c = min(cd, 2)  # f32r needs N>=256 for full rate
                        psc = big_p.tile([128, 512], F32, tag="big", name="psc")
                        nc.tensor.matmul(psc[:, c * 128:512],
                                         kT[:, j * 128:(j + 1) * 128],
                                         qtile[:, c * 128:512],
                                         start=True, stop=True)
                        ex = exp_p.tile([128, 512], F32R)
                        nc.scalar.activation(ex[:, c * 128:512], psc[:, c * 128:512],
                                             AF.Exp, scale=ISQ)
                        if j >= 4 * g:
                            if cd <= 2:
                                nc.vector.tensor_mul(
                                    ex[:, cd * 128:(cd + 1) * 128],
                                    ex[:, cd * 128:(cd + 1) * 128].bitcast(F32),
                                    tri_sb[:])
                            else:
                                nc.vector.tensor_mul(
                                    ex[:, 256:512],
                                    ex[:, 256:512].bitcast(F32),
                                    ztri_sb[:])
                        nc.tensor.matmul(pat[:, c * 128:512], v_sb[:, j, :],
                                         ex[:, c * 128:512],
                                         start=(j == 0), stop=(j == nj - 1),
                                         skip_group_check=True)
                        nc.tensor.matmul(pden[:, c * 128:512], oc_sb[:],
                                         ex[:, c * 128:512],
                                         start=(j == 0), stop=(j == nj - 1),
                                         skip_group_check=True)
                    rc = rc_p.tile([1, 512], F32, tag="rc")
                    nc.vector.reciprocal_approx_fast(rc[:], pden[:])
                    bc = rc_p.tile([128, 512], F32, tag="bc")
                    nc.gpsimd.partition_broadcast(bc[:], rc[:])
                    nc.vector.tensor_tensor(attnT[:, h, g * 512:(g + 1) * 512],
                                            pat[:], bc[:], op=OP.mult)

                # out-proj for this supertile's 4 row blocks
                for i in range(4 * g, 4 * g + 4):
                    ot = ost_p.tile([128, D], F32)
                    for n in range(NOC):
                        po = po_p.tile([128, 512], F32)
                        for h in range(HPC):
                            nc.tensor.matmul(po[:], attnT[:, h, i * 128:(i + 1) * 128],
                                             wo4[:, h, n, :],
                                             start=(h == 0), stop=(h == HPC - 1))
                        if n % 2 == 0:
                            nc.scalar.copy(ot[:, n * 512:(n + 1) * 512], po[:])
                        else:
                            nc.vector.tensor_copy(ot[:, n * 512:(n + 1) * 512], po[:])
                    nc.sync.dma_start(
                        out.ap()[i * 128:(i + 1) * 128, 0:2048], ot[:, 0:2048])
                    nc.sync.dma_start(
                        out.ap()[i * 128:(i + 1) * 128, 2048:4096], ot[:, 2048:4096])


_NC_CACHE = None


def _build():
    global _NC_CACHE
    if _NC_CACHE is None:
        nc = bacc.Bacc("TRN2", target_bir_lowering=False, debug=False)
        with tile.TileContext(nc) as tc:
            _emit(nc, tc)
        nc.compile()
        _NC_CACHE = nc
    return _NC_CACHE


def kernel(x, mask, cos, sin, Wq, Wk, Wv, Wo, qn_w, kn_w):
    x = np.asarray(x, np.float32)
    cos = np.asarray(cos, np.float32)
    sin = np.asarray(sin, np.float32)
    Wq = np.asarray(Wq, np.float32)
    Wk = np.asarray(Wk, np.float32)
    Wv = np.asarray(Wv, np.float32)
    Wo = np.asarray(Wo, np.float32)
    qn_w = np.asarray(qn_w, np.float32)
    kn_w = np.asarray(kn_w, np.float32)

    nc = _build()

    # xt: [NB, 128(d within ktile), DKT*128] blocks of x^T
    xt = np.ascontiguousarray(
        x.T.reshape(DKT, 128, NB, 128).transpose(2, 1, 0, 3)
    ).reshape(NB, 128, DKT * 128)

    # rope tables with rmsnorm sqrt(HD) and q/k norm weights folded in
    sgn = np.concatenate([-np.ones(HD // 2, np.float32),
                          np.ones(HD // 2, np.float32)])
    rt = float(np.sqrt(HD))
    cq = cos * (qn_w * rt)[None, :]
    sq = sin * (sgn * np.roll(qn_w, -(HD // 2)) * rt)[None, :]
    ck = cos * (kn_w * rt)[None, :]
    sk = sin * (sgn * np.roll(kn_w, -(HD // 2)) * rt)[None, :]

    tri_np = np.where(np.arange(128)[:, None] > np.arange(128)[None, :],
                      np.float32(0.0), np.float32(1.0))
    ztri_np = np.concatenate([np.zeros((128, 128), np.float32), tri_np], axis=1)

    tblp = np.ascontiguousarray(
        np.stack([cq, sq, ck, sk]).astype(np.float32))
    base = dict(
        xt=xt, tbl=tblp,
        tri01=tri_np, ztri01=ztri_np,
        onescol=np.ones((128, 1), np.float32),
        ident=np.eye(128, dtype=np.float32),
    )
    in_maps = []
    for cidx in range(NCORES):
        wq_c = Wq[:, cidx * HPC * HD:(cidx + 1) * HPC * HD]
        wq_t = np.ascontiguousarray(
            wq_c.reshape(DKT, 128, HPC * HD).transpose(1, 0, 2)
        ).reshape(128, DKT * HPC * HD)
        wk_c = Wk[:, cidx * HD:(cidx + 1) * HD]
        wv_c = Wv[:, cidx * HD:(cidx + 1) * HD]
        wkv_c = np.concatenate([wk_c, wv_c], axis=1)
        wkv_t = np.ascontiguousarray(
            wkv_c.reshape(DKT, 128, 256).transpose(1, 0, 2)
        ).reshape(128, DKT * 256)
        wo_c = Wo[cidx * HPC * HD:(cidx + 1) * HPC * HD, :]
        wo_t = np.ascontiguousarray(
            wo_c.reshape(HPC, HD, NOC, 512).transpose(1, 0, 2, 3)
        ).reshape(128, HPC * NOC * 512)
        in_maps.append(dict(base, wq=wq_t, wkv=wkv_t, wo=wo_t))

    res = run_bass_kernel_spmd(nc, in_maps, core_ids=list(range(NCORES)))
    acc = res.results[0]["out"].astype(np.float32).copy()
    for r in res.results[1:]:
        acc += r["out"]
    return acc



# revision 2
# speedup vs baseline: 1.0715x; 1.0715x over previous
"""GQA attention kernel for Trainium2, tensor-parallel over heads across 8 cores.

Problem: T=2048, D=4096, H=32 q-heads, G=8 kv-heads, HD=128.
Per core: 4 q heads + 1 kv head (group), full T.
All matmul operands in bf16 (FWL-enabled weight loads, half the HBM
traffic of f32r); psum accumulation stays f32.
  q = rmsnorm(x @ Wq_c) -> rope -> qT [HD, T]
  k = rmsnorm(x @ Wk_c) -> rope -> kT [HD, T]; v natural [T, HD]
  scoresT[j,i] blocks = kT_j.T @ qT, causal-blocked
  expT = exp(scoresT/sqrt(HD) + diag mask); denom via ones-matmul
  attnT[HD, T] += v_j.T @ expT_j; normalized by 1/denom
  out_partial = attnT.T @ Wo_c (bf16, row-sharded) -> host sums the 8 partials.
"""
import sys

sys.path.insert(0, '/opt/trn_rl_repo')

import numpy as np
import ml_dtypes

import concourse.bass as bass
import concourse.bacc as bacc
import concourse.mybir as mybir
import concourse.tile as tile
from concourse.bass_utils import run_bass_kernel_spmd

F32 = mybir.dt.float32
F32R = mybir.dt.float32r
BF16 = mybir.dt.bfloat16
AF = mybir.ActivationFunctionType
OP = mybir.AluOpType

T = 2048
D = 4096
H = 32
G = 8
HD = 128
NCORES = 8
HPC = H // NCORES          # 4 q heads per core
NB = T // 128              # 16 row/col blocks
NSUP = NB // 4             # 4 supertiles of 512 queries
DKT = D // 128             # 32 contraction tiles for projections
NOC = D // 512             # 8 out-proj column blocks
EPS = 1e-6
ISQ = 1.0 / float(np.sqrt(HD))


def _rotview(ap):
    """[128, 128] AP -> [128, 2, 64] view reading cols 64:128 then 0:64."""
    return bass.AP(ap.tensor, ap.offset + 64, [list(ap.ap[0]), [-64, 2], [1, 64]])


def _emit(nc, tc):
    xt = nc.dram_tensor("xt", [NB, 128, DKT * 128], BF16, kind="ExternalInput")
    wq = nc.dram_tensor("wq", [128, DKT * 512], BF16, kind="ExternalInput")
    wkv = nc.dram_tensor("wkv", [128, DKT * 256], BF16, kind="ExternalInput")
    wo = nc.dram_tensor("wo", [128, HPC * NOC * 512], BF16, kind="ExternalInput")
    tbl = nc.dram_tensor("tbl", [4, T, HD], F32, kind="ExternalInput")
    tri01 = nc.dram_tensor("tri01", [128, 128], BF16, kind="ExternalInput")
    onescol = nc.dram_tensor("onescol", [128, 1], BF16, kind="ExternalInput")
    ident = nc.dram_tensor("ident", [128, 128], F32R, kind="ExternalInput")
    out = nc.dram_tensor("out", [T, D], BF16, kind="ExternalOutput")

    import contextlib
    ctx = contextlib.ExitStack()
    with ctx:
        const_p = ctx.enter_context(tc.tile_pool(name="const", bufs=1))
        tri_sb = const_p.tile([128, 128], BF16)
        oc_sb = const_p.tile([128, 1], BF16)
        id_sb = const_p.tile([128, 128], F32R)
        epsb = const_p.tile([128, 1], F32)

        def load_consts():
            nc.sync.dma_start(id_sb[:], ident.ap())
            nc.sync.dma_start(tri_sb[:], tri01.ap())
            nc.sync.dma_start(oc_sb[:], onescol.ap())
            nc.vector.memset(epsb[:], float(HD) * EPS)

        # persistent SBUF: kT, v, attnT (all bf16)
        pers = ctx.enter_context(tc.tile_pool(name="pers", bufs=1))
        kT = pers.tile([128, T], BF16)
        v_sb = pers.tile([128, NB, 128], BF16)
        attnT = pers.tile([128, HPC, T], BF16)

        # DRAM scratch for qT spill
        dram_p = ctx.enter_context(tc.tile_pool(name="dram", bufs=1, space="DRAM"))
        qt_d = dram_p.tile([HPC, 128, T], BF16)

        # right-side psum shared by phase-1 transposes and phase-2 scores
        big_p = ctx.enter_context(
            tc.tile_pool(name="big_p", bufs=3, space="PSUM", side="right"))
        pat_p = ctx.enter_context(
            tc.tile_pool(name="pat_p", bufs=2, space="PSUM", side="right"))

        # ---------------- Phase 1: projections + rmsnorm + rope + transpose ----
        with tc.tile_pool(name="wq_p", bufs=1) as wq_p, \
             tc.tile_pool(name="wkv_p", bufs=1) as wkv_p, \
             tc.tile_pool(name="xb_p", bufs=2) as xb_p, \
             tc.tile_pool(name="tbl_p", bufs=2) as tbl_p, \
             tc.tile_pool(name="scr_p", bufs=2) as scr_p, \
             tc.tile_pool(name="stg_p", bufs=2) as stg_p, \
             tc.tile_pool(name="psq_p", bufs=2, space="PSUM") as psq_p, \
             tc.tile_pool(name="pskv_p", bufs=1, space="PSUM") as pskv_p:
            wq_sb = wq_p.tile([128, DKT * 512], BF16)
            wkv_sb = wkv_p.tile([128, DKT * 256], BF16)
            def load_wq(part):
                if part == 0:
                    nc.sync.dma_start(wq_sb[:, 0:1024], wq.ap()[:, 0:1024])
                    return
                for chh in range(1, 16):
                    w = DKT * 512 // 16
                    nc.sync.dma_start(wq_sb[:, chh * w:(chh + 1) * w],
                                      wq.ap()[:, chh * w:(chh + 1) * w])
                for chh in range(4):
                    w = DKT * 256 // 4
                    nc.sync.dma_start(wkv_sb[:, chh * w:(chh + 1) * w],
                                      wkv.ap()[:, chh * w:(chh + 1) * w])
            wq3 = wq_sb[:].rearrange("p (k n) -> p k n", k=DKT)
            wkv3 = wkv_sb[:].rearrange("p (k n) -> p k n", k=DKT)

            for i in range(NB):
                xb = xb_p.tile([128, DKT * 128], BF16)
                if i == 0:
                    load_wq(0)
                nch = 8 if i == 0 else 2
                for chh in range(nch):
                    w = DKT * 128 // nch
                    nc.sync.dma_start(xb[:, chh * w:(chh + 1) * w],
                                      xt.ap()[i][:, chh * w:(chh + 1) * w])
                xb3 = xb[:].rearrange("p (k n) -> p k n", k=DKT)

                tb = tbl_p.tile([128, 4, HD], F32, tag="tb")
                nc.sync.dma_start(
                    tb[:],
                    tbl.ap()[:, i * 128:(i + 1) * 128, :].transpose([1, 0, 2]))
                cq, sq, ck, sk = (tb[:, 0, :], tb[:, 1, :], tb[:, 2, :],
                                  tb[:, 3, :])
                if i == 0:
                    load_wq(1)
                    load_consts()

                psq = psq_p.tile([128, 512], F32)
                pskv = pskv_p.tile([128, 256], F32)
                for kk in range(DKT):
                    nc.tensor.matmul(psq[:], xb3[:, kk, :], wq3[:, kk, :],
                                     start=(kk == 0), stop=(kk == DKT - 1))
                for kk in range(DKT):
                    nc.tensor.matmul(pskv[:], xb3[:, kk, :], wkv3[:, kk, :],
                                     start=(kk == 0), stop=(kk == DKT - 1))

                # rms stats for 4 q chunks + 1 k chunk
                ssq = scr_p.tile([128, 8], F32, tag="ssq")
                sqscr = scr_p.tile([128, 128], F32, tag="sqscr")
                for c in range(HPC):
                    nc.scalar.activation(sqscr[:], psq[:, c * 128:(c + 1) * 128],
                                         AF.Square, accum_out=ssq[:, c:c + 1])
                nc.scalar.activation(sqscr[:], pskv[:, 0:128],
                                     AF.Square, accum_out=ssq[:, 4:5])
                rstd = scr_p.tile([128, 8], F32, tag="rstd")
                nc.scalar.activation(rstd[:, 0:5], ssq[:, 0:5], AF.Sqrt,
                                     bias=epsb[:])
                nc.vector.reciprocal_approx_fast(rstd[:, 0:5], rstd[:, 0:5])

                # rope (rmsnorm scale fused; sqrt(HD) and qn/kn weights folded
                # into the host-side cos/sin tables)
                roq = scr_p.tile([128, 512], F32R, tag="roq")
                rok = scr_p.tile([128, 128], F32R, tag="rok")
                t1 = scr_p.tile([128, 128], F32, tag="t1")
                t2 = scr_p.tile([128, 128], F32, tag="t2")
                for c in range(HPC):
                    ch = psq[:, c * 128:(c + 1) * 128]
                    nc.vector.scalar_tensor_tensor(
                        t1[:], ch, rstd[:, c:c + 1], cq,
                        op0=OP.mult, op1=OP.mult)
                    nc.vector.scalar_tensor_tensor(
                        t2[:].rearrange("p (a b) -> p a b", a=2), _rotview(ch),
                        rstd[:, c:c + 1], sq.rearrange("p (a b) -> p a b", a=2),
                        op0=OP.mult, op1=OP.mult)
                    nc.vector.tensor_add(roq[:, c * 128:(c + 1) * 128], t1[:], t2[:])
                chk = pskv[:, 0:128]
                nc.vector.scalar_tensor_tensor(
                    t1[:], chk, rstd[:, 4:5], ck, op0=OP.mult, op1=OP.mult)
                nc.vector.scalar_tensor_tensor(
                    t2[:].rearrange("p (a b) -> p a b", a=2), _rotview(chk),
                    rstd[:, 4:5], sk.rearrange("p (a b) -> p a b", a=2),
                    op0=OP.mult, op1=OP.mult)
                nc.vector.tensor_add(rok[:], t1[:], t2[:])

                # v: psum -> sbuf (f32 -> bf16)
                nc.scalar.copy(v_sb[:, i, :], pskv[:, 128:256])

                # transposes: 4 q chunks + 1 k via shared right-side psum pool
                trq = big_p.tile([128, 512], F32, tag="big", name="trq")
                for c in range(HPC):
                    nc.tensor.transpose(
                        trq[:, c * 128:(c + 1) * 128].bitcast(F32R),
                        roq[:, c * 128:(c + 1) * 128], id_sb[:])
                trk = big_p.tile([128, 512], F32, tag="big", name="trk")
                nc.tensor.transpose(trk[:, 0:128].bitcast(F32R), rok[:], id_sb[:])

                qstg = stg_p.tile([128, 512], BF16)
                nc.scalar.copy(qstg[:], trq[:])
                nc.gpsimd.dma_start(
                    qt_d[:, :, i * 128:(i + 1) * 128].transpose([1, 0, 2]),
                    qstg[:].rearrange("p (h n) -> p h n", h=HPC))
                nc.scalar.copy(kT[:, i * 128:(i + 1) * 128], trk[:, 0:128])

        # ---------------- Phase 2+3: attention + out-proj, interleaved --------
        with tc.tile_pool(name="wo_p", bufs=1) as wo_p, \
             tc.tile_pool(name="qt_p", bufs=2, side="right") as qt_p, \
             tc.tile_pool(name="exp_p", bufs=3, side="right") as exp_p, \
             tc.tile_pool(name="rc_p", bufs=2) as rc_p, \
             tc.tile_pool(name="pden_p", bufs=1, space="PSUM") as pden_p, \
             tc.tile_pool(name="po_p", bufs=2, space="PSUM") as po_p, \
             tc.tile_pool(name="ost_p", bufs=2) as ost_p:
            wo_sb = wo_p.tile([128, HPC * NOC * 512], BF16)
            for chh in range(8):
                w = HPC * NOC * 512 // 8
                nc.sync.dma_start(wo_sb[:, chh * w:(chh + 1) * w],
                                  wo.ap()[:, chh * w:(chh + 1) * w])
            wo4 = wo_sb[:].rearrange("p (h n c) -> p h n c", h=HPC, n=NOC)
            for g in range(NSUP):
                for h in range(HPC):
                    qtile = qt_p.tile([128, 512], BF16)
                    nc.gpsimd.dma_start(qtile[:], qt_d[h, :, g * 512:(g + 1) * 512])
                    pat = pat_p.tile([128, 512], F32, tag="pat", name="pat")
                    pden = pden_p.tile([1, 512], F32)
                    nj = 4 * g + 4
                    for j in range(nj):
                        c = max(0, j - 4 * g)
                        psc = big_p.tile([128, 512], F32, tag="big", name="psc")
                        nc.tensor.matmul(psc[:, c * 128:512],
                                         kT[:, j * 128:(j + 1) * 128],
                                         qtile[:, c * 128:512],
                                         start=True, stop=True)
                        ex = exp_p.tile([128, 512], BF16)
                        nc.scalar.activation(ex[:, c * 128:512], psc[:, c * 128:512],
                                             AF.Exp, scale=ISQ)
                        if j >= 4 * g:
                            nc.vector.tensor_mul(
                                ex[:, c * 128:(c + 1) * 128],
                                ex[:, c * 128:(c + 1) * 128],
                                tri_sb[:])
                        nc.tensor.matmul(pat[:, c * 128:512], v_sb[:, j, :],
                                         ex[:, c * 128:512],
                                         start=(j == 0), stop=(j == nj - 1),
                                         skip_group_check=True)
                        nc.tensor.matmul(pden[:, c * 128:512], oc_sb[:],
                                         ex[:, c * 128:512],
                                         start=(j == 0), stop=(j == nj - 1),
                                         skip_group_check=True)
                    rc = rc_p.tile([1, 512], F32, tag="rc")
                    nc.vector.reciprocal_approx_fast(rc[:], pden[:])
                    bc = rc_p.tile([128, 512], F32, tag="bc")
                    nc.gpsimd.partition_broadcast(bc[:], rc[:])
                    nc.vector.tensor_tensor(attnT[:, h, g * 512:(g + 1) * 512],
                                            pat[:], bc[:], op=OP.mult)

                # out-proj for this supertile's 4 row blocks
                for i in range(4 * g, 4 * g + 4):
                    ot = ost_p.tile([128, D], BF16)
                    for n in range(NOC):
                        po = po_p.tile([128, 512], F32)
                        for h in range(HPC):
                            nc.tensor.matmul(po[:], attnT[:, h, i * 128:(i + 1) * 128],
                                             wo4[:, h, n, :],
                                             start=(h == 0), stop=(h == HPC - 1))
                        if n % 2 == 0:
                            nc.scalar.copy(ot[:, n * 512:(n + 1) * 512], po[:])
                        else:
                            nc.vector.tensor_copy(ot[:, n * 512:(n + 1) * 512], po[:])
                    nc.sync.dma_start(
                        out.ap()[i * 128:(i + 1) * 128, 0:2048], ot[:, 0:2048])
                    nc.sync.dma_start(
                        out.ap()[i * 128:(i + 1) * 128, 2048:4096], ot[:, 2048:4096])


_NC_CACHE = None


def _build():
    global _NC_CACHE
    if _NC_CACHE is None:
        nc = bacc.Bacc("TRN2", target_bir_lowering=False, debug=False)
        with tile.TileContext(nc) as tc:
            _emit(nc, tc)
        nc.compile()
        _NC_CACHE = nc
    return _NC_CACHE


def kernel(x, mask, cos, sin, Wq, Wk, Wv, Wo, qn_w, kn_w):
    x = np.asarray(x, np.float32)
    cos = np.asarray(cos, np.float32)
    sin = np.asarray(sin, np.float32)
    Wq = np.asarray(Wq, np.float32)
    Wk = np.asarray(Wk, np.float32)
    Wv = np.asarray(Wv, np.float32)
    Wo = np.asarray(Wo, np.float32)
    qn_w = np.asarray(qn_w, np.float32)
    kn_w = np.asarray(kn_w, np.float32)

    nc = _build()

    BF = ml_dtypes.bfloat16

    # xt: [NB, 128(d within ktile), DKT*128] blocks of x^T
    xt = np.ascontiguousarray(
        x.T.reshape(DKT, 128, NB, 128).transpose(2, 1, 0, 3)
    ).reshape(NB, 128, DKT * 128).astype(BF)

    # rope tables with rmsnorm sqrt(HD) and q/k norm weights folded in
    sgn = np.concatenate([-np.ones(HD // 2, np.float32),
                          np.ones(HD // 2, np.float32)])
    rt = float(np.sqrt(HD))
    cq = cos * (qn_w * rt)[None, :]
    sq = sin * (sgn * np.roll(qn_w, -(HD // 2)) * rt)[None, :]
    ck = cos * (kn_w * rt)[None, :]
    sk = sin * (sgn * np.roll(kn_w, -(HD // 2)) * rt)[None, :]

    tri_np = np.where(np.arange(128)[:, None] > np.arange(128)[None, :],
                      np.float32(0.0), np.float32(1.0))

    tblp = np.ascontiguousarray(
        np.stack([cq, sq, ck, sk]).astype(np.float32))
    base = dict(
        xt=xt, tbl=tblp,
        tri01=tri_np.astype(BF),
        onescol=np.ones((128, 1), BF),
        ident=np.eye(128, dtype=np.float32),
    )
    in_maps = []
    for cidx in range(NCORES):
        wq_c = Wq[:, cidx * HPC * HD:(cidx + 1) * HPC * HD]
        wq_t = np.ascontiguousarray(
            wq_c.reshape(DKT, 128, HPC * HD).transpose(1, 0, 2)
        ).reshape(128, DKT * HPC * HD).astype(BF)
        wk_c = Wk[:, cidx * HD:(cidx + 1) * HD]
        wv_c = Wv[:, cidx * HD:(cidx + 1) * HD]
        wkv_c = np.concatenate([wk_c, wv_c], axis=1)
        wkv_t = np.ascontiguousarray(
            wkv_c.reshape(DKT, 128, 256).transpose(1, 0, 2)
        ).reshape(128, DKT * 256).astype(BF)
        wo_c = Wo[cidx * HPC * HD:(cidx + 1) * HPC * HD, :]
        wo_t = np.ascontiguousarray(
            wo_c.reshape(HPC, HD, NOC, 512).transpose(1, 0, 2, 3)
        ).reshape(128, HPC * NOC * 512).astype(BF)
        in_maps.append(dict(base, wq=wq_t, wkv=wkv_t, wo=wo_t))

    res = run_bass_kernel_spmd(nc, in_maps, core_ids=list(range(NCORES)))
    acc = res.results[0]["out"].astype(np.float32)
    for r in res.results[1:]:
        acc = acc + r["out"].astype(np.float32)
    return acc


# revision 14
# speedup vs baseline: 1.1293x; 1.0540x over previous
"""GQA attention kernel for Trainium2, tensor-parallel over heads across 8 cores.

Problem: T=2048, D=4096, H=32 q-heads, G=8 kv-heads, HD=128.
Per core: 4 q heads + 1 kv head (group), full T.
All matmul operands bf16 (FWL weight loads, half the HBM traffic); psum f32.
  q = rmsnorm(x @ Wq_c) -> rope -> qT [HD, h, T] kept in SBUF
  k = rmsnorm(x @ Wk_c) -> rope -> kT [HD, T]; v natural [T, HD]
  scoresT[j,i] blocks = kT_j.T @ qT (causal-blocked)
  expT = exp(scoresT/sqrt(HD)) * diag mask
  denom: DVE accumulates den_acc[128,T-block] += expT_j, one gpsimd
         partition_all_reduce per (g,h) (no tensor-engine ones-matmuls)
  attnT[HD, T] += v_j.T @ expT_j; normalized by 1/denom
  out_partial = attnT.T @ Wo_c (bf16, row-sharded) -> host sums the 8 partials.
"""
import sys

sys.path.insert(0, '/opt/trn_rl_repo')

import numpy as np
import ml_dtypes

import concourse.bass as bass
import concourse.bacc as bacc
import concourse.mybir as mybir
import concourse.tile as tile
from concourse.bass_isa import ReduceOp
from concourse.bass_utils import run_bass_kernel_spmd

F32 = mybir.dt.float32
F32R = mybir.dt.float32r
BF16 = mybir.dt.bfloat16
AF = mybir.ActivationFunctionType
OP = mybir.AluOpType

T = 2048
D = 4096
H = 32
G = 8
HD = 128
NCORES = 8
HPC = H // NCORES          # 4 q heads per core
NB = T // 128              # 16 row/col blocks
NSUP = NB // 4             # 4 supertiles of 512 queries
DKT = D // 128             # 32 contraction tiles for projections
NOC = D // 512             # 8 out-proj column blocks
EPS = 1e-6
ISQ = 1.0 / float(np.sqrt(HD))


def _rotview(ap):
    """[128, 128] AP -> [128, 2, 64] view reading cols 64:128 then 0:64."""
    return bass.AP(ap.tensor, ap.offset + 64, [list(ap.ap[0]), [-64, 2], [1, 64]])


def _emit(nc, tc):
    xt = nc.dram_tensor("xt", [NB, 128, DKT * 128], BF16, kind="ExternalInput")
    wq = nc.dram_tensor("wq", [128, DKT * 512], BF16, kind="ExternalInput")
    wkv = nc.dram_tensor("wkv", [128, DKT * 256], BF16, kind="ExternalInput")
    wo = nc.dram_tensor("wo", [128, HPC * NOC * 512], BF16, kind="ExternalInput")
    tbl = nc.dram_tensor("tbl", [4, T, HD], F32, kind="ExternalInput")
    tri01 = nc.dram_tensor("tri01", [128, 128], BF16, kind="ExternalInput")
    ones128 = nc.dram_tensor("ones128", [128, 128], BF16, kind="ExternalInput")
    ident = nc.dram_tensor("ident", [128, 128], F32R, kind="ExternalInput")
    out = nc.dram_tensor("out", [T, D], BF16, kind="ExternalOutput")

    import contextlib
    ctx = contextlib.ExitStack()
    with ctx:
        const_p = ctx.enter_context(tc.tile_pool(name="const", bufs=1))
        tri_sb = const_p.tile([128, 128], BF16)
        oc_sb = const_p.tile([128, 128], BF16)
        id_sb = const_p.tile([128, 128], F32R)
        epsb = const_p.tile([128, 1], F32)

        def load_consts():
            nc.sync.dma_start(id_sb[:], ident.ap())
            nc.sync.dma_start(tri_sb[:], tri01.ap())
            nc.sync.dma_start(oc_sb[:], ones128.ap())
            nc.vector.memset(epsb[:], float(HD) * EPS)

        # persistent SBUF: kT, v, qT, attnT, Wo (all bf16)
        pers = ctx.enter_context(tc.tile_pool(name="pers", bufs=1))
        kT = pers.tile([128, T], BF16)
        v_sb = pers.tile([128, NB, 128], BF16)
        qT_sb = pers.tile([128, HPC, T], BF16)
        attnT = pers.tile([128, HPC, T], BF16)
        wo_sb = pers.tile([128, HPC * NOC * 512], BF16)
        wo4 = wo_sb[:].rearrange("p (h n c) -> p h n c", h=HPC, n=NOC)

        # right-side psum shared by phase-1 transposes and phase-2 scores
        big_p = ctx.enter_context(
            tc.tile_pool(name="big_p", bufs=3, space="PSUM", side="right"))
        pat_p = ctx.enter_context(
            tc.tile_pool(name="pat_p", bufs=2, space="PSUM", side="right"))

        # ---------------- Phase 1: projections + rmsnorm + rope + transpose ----
        with tc.tile_pool(name="wq_p", bufs=1) as wq_p, \
             tc.tile_pool(name="wkv_p", bufs=1) as wkv_p, \
             tc.tile_pool(name="xb_p", bufs=2) as xb_p, \
             tc.tile_pool(name="tbl_p", bufs=2) as tbl_p, \
             tc.tile_pool(name="scr_p", bufs=2) as scr_p, \
             tc.tile_pool(name="psq_p", bufs=2, space="PSUM") as psq_p, \
             tc.tile_pool(name="pskv_p", bufs=1, space="PSUM") as pskv_p:
            wq_sb = wq_p.tile([128, DKT * 512], BF16)
            wkv_sb = wkv_p.tile([128, DKT * 256], BF16)
            def load_wq(part):
                if part == 0:
                    nc.sync.dma_start(wq_sb[:, 0:512], wq.ap()[:, 0:512])
                    return
                for chh in range(1, 32):
                    w = 512
                    nc.sync.dma_start(wq_sb[:, chh * w:(chh + 1) * w],
                                      wq.ap()[:, chh * w:(chh + 1) * w])
                for chh in range(4):
                    w = DKT * 256 // 4
                    nc.sync.dma_start(wkv_sb[:, chh * w:(chh + 1) * w],
                                      wkv.ap()[:, chh * w:(chh + 1) * w])
            wq3 = wq_sb[:].rearrange("p (k n) -> p k n", k=DKT)
            wkv3 = wkv_sb[:].rearrange("p (k n) -> p k n", k=DKT)

            for i in range(NB):
                xb = xb_p.tile([128, DKT * 128], BF16)
                if i == 0:
                    load_wq(0)
                nch = 16 if i == 0 else 2
                for chh in range(nch):
                    w = DKT * 128 // nch
                    nc.sync.dma_start(xb[:, chh * w:(chh + 1) * w],
                                      xt.ap()[i][:, chh * w:(chh + 1) * w])
                xb3 = xb[:].rearrange("p (k n) -> p k n", k=DKT)

                tb = tbl_p.tile([128, 4, HD], F32, tag="tb")
                nc.sync.dma_start(
                    tb[:],
                    tbl.ap()[:, i * 128:(i + 1) * 128, :].transpose([1, 0, 2]))
                cq, sq, ck, sk = (tb[:, 0, :], tb[:, 1, :], tb[:, 2, :],
                                  tb[:, 3, :])
                if i == 0:
                    load_wq(1)
                    load_consts()
                if i == 1:
                    # preload Wo while projections stream
                    for chh in range(8):
                        w = HPC * NOC * 512 // 8
                        nc.sync.dma_start(wo_sb[:, chh * w:(chh + 1) * w],
                                          wo.ap()[:, chh * w:(chh + 1) * w])

                psq = psq_p.tile([128, 512], F32)
                pskv = pskv_p.tile([128, 256], F32)
                for kk in range(DKT):
                    nc.tensor.matmul(psq[:], xb3[:, kk, :], wq3[:, kk, :],
                                     start=(kk == 0), stop=(kk == DKT - 1))
                for kk in range(DKT):
                    nc.tensor.matmul(pskv[:], xb3[:, kk, :], wkv3[:, kk, :],
                                     start=(kk == 0), stop=(kk == DKT - 1))

                # rms stats for 4 q chunks + 1 k chunk
                ssq = scr_p.tile([128, 8], F32, tag="ssq")
                sqscr = scr_p.tile([128, 128], F32, tag="sqscr")
                for c in range(HPC):
                    nc.scalar.activation(sqscr[:], psq[:, c * 128:(c + 1) * 128],
                                         AF.Square, accum_out=ssq[:, c:c + 1])
                nc.scalar.activation(sqscr[:], pskv[:, 0:128],
                                     AF.Square, accum_out=ssq[:, 4:5])
                rstd = scr_p.tile([128, 8], F32, tag="rstd")
                nc.scalar.activation(rstd[:, 0:5], ssq[:, 0:5], AF.Sqrt,
                                     bias=epsb[:])
                nc.vector.reciprocal_approx_fast(rstd[:, 0:5], rstd[:, 0:5])

                # rope (rmsnorm scale fused; sqrt(HD) and qn/kn weights folded
                # into the host-side cos/sin tables); bf16 outputs
                roq = scr_p.tile([128, 512], F32R, tag="roq")
                rok = scr_p.tile([128, 128], F32R, tag="rok")
                t1 = scr_p.tile([128, 128], F32, tag="t1")
                t2 = scr_p.tile([128, 128], F32, tag="t2")
                for c in range(HPC):
                    ch = psq[:, c * 128:(c + 1) * 128]
                    nc.vector.scalar_tensor_tensor(
                        t1[:], ch, rstd[:, c:c + 1], cq,
                        op0=OP.mult, op1=OP.mult)
                    nc.vector.scalar_tensor_tensor(
                        t2[:].rearrange("p (a b) -> p a b", a=2), _rotview(ch),
                        rstd[:, c:c + 1], sq.rearrange("p (a b) -> p a b", a=2),
                        op0=OP.mult, op1=OP.mult)
                    nc.vector.tensor_add(roq[:, c * 128:(c + 1) * 128], t1[:], t2[:])
                chk = pskv[:, 0:128]
                nc.vector.scalar_tensor_tensor(
                    t1[:], chk, rstd[:, 4:5], ck, op0=OP.mult, op1=OP.mult)
                nc.vector.scalar_tensor_tensor(
                    t2[:].rearrange("p (a b) -> p a b", a=2), _rotview(chk),
                    rstd[:, 4:5], sk.rearrange("p (a b) -> p a b", a=2),
                    op0=OP.mult, op1=OP.mult)
                nc.vector.tensor_add(rok[:], t1[:], t2[:])

                # v: psum -> sbuf (f32 -> bf16)
                nc.scalar.copy(v_sb[:, i, :], pskv[:, 128:256])

                # transposes (f32r): 4 q chunks + 1 k via shared psum pool
                trq = big_p.tile([128, 512], F32, tag="big", name="trq")
                for c in range(HPC):
                    nc.tensor.transpose(
                        trq[:, c * 128:(c + 1) * 128].bitcast(F32R),
                        roq[:, c * 128:(c + 1) * 128], id_sb[:])
                trk = big_p.tile([128, 512], F32, tag="big", name="trk")
                nc.tensor.transpose(trk[:, 0:128].bitcast(F32R), rok[:], id_sb[:])

                # qT stays in SBUF (no DRAM spill); copies convert f32 -> bf16
                for c in range(HPC):
                    nc.scalar.copy(qT_sb[:, c, i * 128:(i + 1) * 128],
                                   trq[:, c * 128:(c + 1) * 128])
                nc.scalar.copy(kT[:, i * 128:(i + 1) * 128], trk[:, 0:128])

        # ---------------- Phase 2+3: attention + out-proj, interleaved --------
        with tc.tile_pool(name="exp_p", bufs=3, side="right") as exp_p, \
             tc.tile_pool(name="den_p", bufs=2) as den_p, \
             tc.tile_pool(name="rc_p", bufs=2) as rc_p, \
             tc.tile_pool(name="pden_p", bufs=1, space="PSUM") as pden_p, \
             tc.tile_pool(name="po_p", bufs=2, space="PSUM") as po_p, \
             tc.tile_pool(name="ost_p", bufs=2) as ost_p:
            pending = None   # deferred normalize: (gh_slice, pat, bc)

            def emit_normalize():
                nonlocal pending
                if pending is not None:
                    sl, ppat, pbc = pending
                    nc.vector.tensor_tensor(sl, ppat, pbc, op=OP.mult)
                    pending = None

            for g in range(NSUP):
                for h in range(HPC):
                    qtile = qT_sb[:, h, g * 512:(g + 1) * 512]
                    pat = pat_p.tile([128, 512], F32, tag="pat", name="pat")
                    den = den_p.tile([128, 512], BF16, tag="den")
                    nj = 4 * g + 4
                    for j in range(nj):
                        c = max(0, j - 4 * g)
                        psc = big_p.tile([128, 512], F32, tag="big", name="psc")
                        nc.tensor.matmul(psc[:, c * 128:512],
                                         kT[:, j * 128:(j + 1) * 128],
                                         qtile[:, c * 128:512],
                                         start=True, stop=True)
                        ex = exp_p.tile([128, 512], BF16)
                        nc.scalar.activation(ex[:, c * 128:512], psc[:, c * 128:512],
                                             AF.Exp, scale=ISQ)
                        if j >= 4 * g:
                            nc.vector.tensor_mul(
                                ex[:, c * 128:(c + 1) * 128],
                                ex[:, c * 128:(c + 1) * 128],
                                tri_sb[:])
                        nc.tensor.matmul(pat[:, c * 128:512], v_sb[:, j, :],
                                         ex[:, c * 128:512],
                                         start=(j == 0), stop=(j == nj - 1),
                                         skip_group_check=True)
                        # denominator accumulation on DVE (bf16, 2x mode)
                        if j == 0:
                            nc.vector.tensor_copy(den[:], ex[:, 0:512])
                        else:
                            nc.vector.tensor_add(den[:, c * 128:512],
                                                 den[:, c * 128:512],
                                                 ex[:, c * 128:512])
                        if j == 1:
                            emit_normalize()
                    # single partition-sum matmul per (g,h); ones[128,128]
                    # stationary puts the denominator on every partition
                    psd = pden_p.tile([128, 512], F32)
                    nc.tensor.matmul(psd[:], oc_sb[:], den[:],
                                     start=True, stop=True)
                    bc = rc_p.tile([128, 512], F32, tag="bc")
                    nc.vector.reciprocal_approx_fast(bc[:], psd[:])
                    pending = (attnT[:, h, g * 512:(g + 1) * 512], pat[:], bc[:])
                emit_normalize()

                # out-proj for this supertile's 4 row blocks
                for i in range(4 * g, 4 * g + 4):
                    ot = ost_p.tile([128, D], BF16)
                    for n in range(NOC):
                        po = po_p.tile([128, 512], F32)
                        for h in range(HPC):
                            nc.tensor.matmul(po[:], attnT[:, h, i * 128:(i + 1) * 128],
                                             wo4[:, h, n, :],
                                             start=(h == 0), stop=(h == HPC - 1))
                        if n % 2 == 0:
                            nc.scalar.copy(ot[:, n * 512:(n + 1) * 512], po[:])
                        else:
                            nc.vector.tensor_copy(ot[:, n * 512:(n + 1) * 512], po[:])
                    nc.sync.dma_start(
                        out.ap()[i * 128:(i + 1) * 128, 0:2048], ot[:, 0:2048])
                    nc.sync.dma_start(
                        out.ap()[i * 128:(i + 1) * 128, 2048:4096], ot[:, 2048:4096])


_NC_CACHE = None


def _build():
    global _NC_CACHE
    if _NC_CACHE is None:
        nc = bacc.Bacc("TRN2", target_bir_lowering=False, debug=False)
        with tile.TileContext(nc) as tc:
            _emit(nc, tc)
        nc.compile()
        _NC_CACHE = nc
    return _NC_CACHE


def kernel(x, mask, cos, sin, Wq, Wk, Wv, Wo, qn_w, kn_w):
    x = np.asarray(x, np.float32)
    cos = np.asarray(cos, np.float32)
    sin = np.asarray(sin, np.float32)
    Wq = np.asarray(Wq, np.float32)
    Wk = np.asarray(Wk, np.float32)
    Wv = np.asarray(Wv, np.float32)
    Wo = np.asarray(Wo, np.float32)
    qn_w = np.asarray(qn_w, np.float32)
    kn_w = np.asarray(kn_w, np.float32)

    nc = _build()

    BF = ml_dtypes.bfloat16

    # xt: [NB, 128(d within ktile), DKT*128] blocks of x^T
    xt = np.ascontiguousarray(
        x.T.reshape(DKT, 128, NB, 128).transpose(2, 1, 0, 3)
    ).reshape(NB, 128, DKT * 128).astype(BF)

    # rope tables with rmsnorm sqrt(HD) and q/k norm weights folded in
    sgn = np.concatenate([-np.ones(HD // 2, np.float32),
                          np.ones(HD // 2, np.float32)])
    rt = float(np.sqrt(HD))
    cq = cos * (qn_w * rt)[None, :]
    sq = sin * (sgn * np.roll(qn_w, -(HD // 2)) * rt)[None, :]
    ck = cos * (kn_w * rt)[None, :]
    sk = sin * (sgn * np.roll(kn_w, -(HD // 2)) * rt)[None, :]

    tri_np = np.where(np.arange(128)[:, None] > np.arange(128)[None, :],
                      np.float32(0.0), np.float32(1.0))

    tblp = np.ascontiguousarray(
        np.stack([cq, sq, ck, sk]).astype(np.float32))
    base = dict(
        xt=xt, tbl=tblp,
        tri01=tri_np.astype(BF),
        ones128=np.ones((128, 128), BF),
        ident=np.eye(128, dtype=np.float32),
    )
    in_maps = []
    for cidx in range(NCORES):
        wq_c = Wq[:, cidx * HPC * HD:(cidx + 1) * HPC * HD]
        wq_t = np.ascontiguousarray(
            wq_c.reshape(DKT, 128, HPC * HD).transpose(1, 0, 2)
        ).reshape(128, DKT * HPC * HD).astype(BF)
        wk_c = Wk[:, cidx * HD:(cidx + 1) * HD]
        wv_c = Wv[:, cidx * HD:(cidx + 1) * HD]
        wkv_c = np.concatenate([wk_c, wv_c], axis=1)
        wkv_t = np.ascontiguousarray(
            wkv_c.reshape(DKT, 128, 256).transpose(1, 0, 2)
        ).reshape(128, DKT * 256).astype(BF)
        wo_c = Wo[cidx * HPC * HD:(cidx + 1) * HPC * HD, :]
        wo_t = np.ascontiguousarray(
            wo_c.reshape(HPC, HD, NOC, 512).transpose(1, 0, 2, 3)
        ).reshape(128, HPC * NOC * 512).astype(BF)
        in_maps.append(dict(base, wq=wq_t, wkv=wkv_t, wo=wo_t))

    res = run_bass_kernel_spmd(nc, in_maps, core_ids=list(range(NCORES)))
    acc = res.results[0]["out"].astype(np.float32)
    for r in res.results[1:]:
        acc = acc + r["out"].astype(np.float32)
    return acc
